# revision 1
# baseline (speedup 1.0000x reference)
"""DiffLBP soft-histogram kernel for Trainium2 (8 NeuronCores).

Math: the per-pixel softmax over 256 LBP patterns factorizes exactly into a
product of 8 independent Bernoullis with q_p = 1/2 (1 + z_p),
z_p = tanh((oh/2)*tanh(dh*d_p)).  The histogram is a 16x16 Gram matrix of
z-monomials (4 low bits x 4 high bits) pushed through a constant Walsh +-1
transform (host).  Antipodal offsets give z_{p+4}(r,c) = -z_p((r,c)+off), so
only 4 z planes are computed; the "hi" side uses shifted copies (row shift via
SBUF-SBUF DMA, column shift via AP offset), signs folded into the host
transform.

Device program per core (one batch b, one 255-row half):
  phase 1 (x2 z-tiles of 128 rows):
    one combined DMA loads XA/XB row-windows; GPSIMD computes the 4 diffs;
    ScalarE does the two tanh passes -> z planes (bf16).
  phase 2 (x2):
    mlo tile (chunk-major [128, 64 chunks, 16 planes x 8 cols]) gets the z
    singles + 11 composite monomial planes (batched DVE multiplies);
    mhi tile (plane-major) gets row+col shifted z singles via 3 DMAs, then
    its own composites; Gram accumulates 2x64 matmuls (K=127) into PSUM.
  The center row straddling the two z-tiles (local row 127) is excluded
  (K=127) and fixed on the host from two exported raw z rows.
"""

import os
import numpy as np
from contextlib import ExitStack

H = W = 512
HP = WP = 510          # valid center rows/cols
NROWS_SLICE = 257      # input rows per core slice
NHALF = 255            # center rows per core

# plane index -> subset bitmask of {z0,z1,z2,z3}
PLANE_SUBSET = [0b0000,
                0b0001, 0b0010, 0b0100, 0b1000,       # z0 z1 z2 z3
                0b0011, 0b0110, 0b1100,               # m01 m12 m23
                0b0101, 0b1010,                       # m02 m13
                0b1001,                               # m03
                0b0111, 0b1110,                       # m012 m123
                0b1011, 0b1101,                       # m013 m023
                0b1111]                               # m0123

_PROGRAM_CACHE = {}
last_results = None  # BassKernelResults of the most recent run (for test harness)


def _products(nc, m, R):
    """Emit the 11 composite monomial planes from singles (planes 1..4) of a
    16-plane view m[[part], 16, ...]; writes planes 5..15 on R partitions."""
    nc.vector.tensor_mul(m[:R, 5:8], m[:R, 1:4], m[:R, 2:5])
    nc.vector.tensor_mul(m[:R, 8:10], m[:R, 1:3], m[:R, 3:5])
    nc.vector.tensor_mul(m[:R, 10:11], m[:R, 1:2], m[:R, 4:5])
    nc.vector.tensor_mul(m[:R, 11:13], m[:R, 5:7], m[:R, 3:5])
    nc.vector.tensor_mul(m[:R, 13:15], m[:R, 5:8:2], m[:R, 4:0:-3])
    nc.vector.tensor_mul(m[:R, 15:16], m[:R, 5:6], m[:R, 7:8])


def _build_program(dh: float, oh: float):
    import concourse.bacc as bacc
    import concourse.tile as tile
    from concourse import mybir
    import concourse.bass as bass

    f32 = mybir.dt.float32
    bf16 = mybir.dt.bfloat16
    Tanh = mybir.ActivationFunctionType.Tanh

    nc = bacc.Bacc("TRN2", target_bir_lowering=False, debug=False)
    xs_t = nc.dram_tensor("xs", [NROWS_SLICE, W], f32, kind="ExternalInput")
    xs = xs_t.ap()
    gram = nc.dram_tensor("gram", [128, 128], f32, kind="ExternalOutput").ap()
    # raw boundary z rows (plane-major [4, W]): z-tile0 row 127, z-tile1 row 0
    zedge = nc.dram_tensor("zedge", [2, 4, W], bf16, kind="ExternalOutput").ap()

    with tile.TileContext(nc) as tc, ExitStack() as ctx:
        xpool = ctx.enter_context(tc.tile_pool(name="x", bufs=2))
        dpool = ctx.enter_context(tc.tile_pool(name="d", bufs=2))
        tpool = ctx.enter_context(tc.tile_pool(name="t", bufs=2))
        zpool = ctx.enter_context(tc.tile_pool(name="z", bufs=1))
        mpool = ctx.enter_context(tc.tile_pool(name="m", bufs=1))
        ppool = ctx.enter_context(
            tc.tile_pool(name="ps", bufs=1, space=bass.MemorySpace.PSUM))
        drpool = ctx.enter_context(
            tc.tile_pool(name="dr", bufs=1, space=bass.MemorySpace.DRAM))

        # trigger the tanh ACT table load immediately (overlaps the X DMAs)
        warm = mpool.tile([1, 8], f32, tag="warm")
        nc.vector.memset(warm[:, :], 0.0)
        nc.scalar.activation(warm[:, :], warm[:, :], Tanh)

        # ---- per-tile pipeline ----
        # DRAM bounce for the row-shifted copies (SBUF->SBUF DMA is
        # descriptor-bound at ~50ns/row; DRAM<->SBUF is fast)
        zd = drpool.tile([256, 4, W], bf16, tag="zd")
        ps = ppool.tile([128, 128], f32, tag="ps")
        zsrc, mlos, mhis = [], [], []

        def z_phase(i):
            r0 = 128 * i
            # one DMA: xt[:, j, :] = xs[r0 + j + rl, :] for j in (0, 1)
            xt = xpool.tile([128, 2, W], f32, tag="xt")
            src = bass.AP(xs_t, r0 * W, [[W, 128], [W, 2], [1, W]])
            nc.sync.dma_start(xt[:], src)
            xa, xb = xt[:, 0, :], xt[:, 1, :]

            d = dpool.tile([128, 4, W], f32, tag="d")
            nc.vector.memset(d[:, 0, 0:1], 0.0)
            nc.vector.memset(d[:, 2:4, 511:512], 0.0)
            # d_p[cl] = X_{dy}[cl+dx] - XB[cl]   (cl = x-col = center_col + 1)
            nc.vector.tensor_sub(d[:, 0, 1:512], xa[:, 0:511], xb[:, 1:512])   # (-1,-1)
            nc.vector.tensor_sub(d[:, 1, 0:512], xa[:, 0:512], xb[:, 0:512])   # (-1, 0)
            nc.vector.tensor_sub(d[:, 2, 0:511], xa[:, 1:512], xb[:, 0:511])   # (-1,+1)
            nc.vector.tensor_sub(d[:, 3, 0:511], xb[:, 1:512], xb[:, 0:511])   # ( 0,+1)

            t = tpool.tile([128, 4, W], f32, tag="t")
            nc.scalar.activation(t[:, :, :], d[:, :, :], Tanh, scale=float(dh))
            z = zpool.tile([128, 4, W], bf16, tag=f"zsrc{i}")
            nc.scalar.activation(z[:, :, :], t[:, :, :], Tanh, scale=float(oh) / 2.0)
            zsrc.append(z)
            nc.scalar.dma_start(zd[128 * i:128 * (i + 1), :, :], z[:, :, :])

        def mono_phase(i):
            mlo = mpool.tile([128, 64, 128], bf16, tag=f"mlo{i}")
            ml = mlo[:].rearrange("k g (s c) -> k s g c", c=8)  # [128,16,64,8]
            mlos.append(mlo)
            mhi = mpool.tile([128, 16, W], bf16, tag=f"mhi{i}")
            mhis.append(mhi)
            zr = zsrc[i][:].rearrange("k q (g c) -> k q g c", c=8)
            nc.vector.tensor_copy(ml[:, 1:5, :, :], zr[:, :, :, :])
            # hi singles from the DRAM bounce (own-tile rows only; the center
            # row straddling the tiles is excluded via K=127 + host fix):
            # SA_p[rl, cl] = z_p[rl + dy'_p, cl + dx'_p], (dy', dx') antipodal:
            # p=0..2 -> (+1, +1/0/-1) from zd rows 128i+1..128i+127,
            # p=3 -> (0, -1) from zd rows 128i..128i+126.
            base = zd[128 * i + 1, 0, 2]
            src3 = bass.AP(base.tensor, base.offset,
                           [[4 * W, 127], [W - 1, 3], [1, 510]])
            nc.sync.dma_start(mhi[0:127, 1:4, 1:511], src3)
            nc.scalar.dma_start(mhi[0:127, 4, 1:511],
                                zd[128 * i:128 * i + 127, 3, 0:510])
            nc.vector.memset(mhi[:, 0, :], 1.0)
            nc.vector.memset(mhi[:, 1:5, 0:512:511], 0.0)
            # lo side: ones + edge zeroing (singles already written by ACT)
            nc.vector.memset(ml[:, 0, :, :], 1.0)
            nc.vector.memset(mlo[:, 0, 8:40:8], 0.0)    # cl=0   (g=0, c=0)
            nc.vector.memset(mlo[:, 63, 15:47:8], 0.0)  # cl=511 (g=63, c=7)
            # composites (mhi row 127 unwritten -> restrict)
            _products(nc, ml, 128)
            _products(nc, mhi, 127)

        def gram_phase(i):
            for g in range(64):
                nc.tensor.matmul(
                    ps[:, :],
                    mlos[i][0:127, g, :],
                    mhis[i][0:127, :, 8 * g:8 * g + 8],
                    start=(i == 0 and g == 0),
                    stop=(i == 1 and g == 63),
                )

        z_phase(0)
        z_phase(1)
        # boundary exports (host computes the straddling center row 127)
        nc.scalar.dma_start(zedge[0, :, :], zsrc[0][127:128, :, :])
        nc.scalar.dma_start(zedge[1, :, :], zsrc[1][0:1, :, :])
        mono_phase(0)
        gram_phase(0)
        mono_phase(1)
        gram_phase(1)

        gout = mpool.tile([128, 128], f32, tag="gout")
        nc.vector.tensor_copy(gout[:, :], ps[:, :])
        nc.sync.dma_start(gram[:, :], gout[:, :])

    nc.compile()
    return nc


def _walsh16():
    sgn = 2.0 * ((np.arange(16)[:, None] >> np.arange(4)[None, :]) & 1) - 1.0
    w = np.ones((16, 16))
    for k in range(16):
        for s in range(16):
            v = 1.0
            for p in range(4):
                if s >> p & 1:
                    v *= sgn[k, p]
            w[k, s] = v
    return w


def _monomials16(z4):
    """z4: [4, n] -> [16, n] monomials in subset-index order."""
    n = z4.shape[1]
    m = np.ones((16, n))
    for s in range(16):
        for p in range(4):
            if s >> p & 1:
                m[s] *= z4[p]
    return m


def _postprocess(grams, zedges):
    """grams: 8x [128,128] f32; zedges: 8x [2,4,512] bf16 -> [4,256,1,1]."""
    perm = np.argsort(PLANE_SUBSET)  # subset-index -> plane-index
    w16 = _walsh16()
    hi_sign = np.array([(-1.0) ** bin(s).count("1") for s in range(16)])
    out = np.zeros((4, 256), np.float64)
    for b in range(4):
        g16 = np.zeros((16, 16))
        for half in range(2):
            core = 2 * b + half
            gr = grams[core].astype(np.float64).reshape(16, 8, 16, 8)
            g = np.einsum("sgtg->st", gr)          # sum the 8 diagonal blocks
            g16 += g[np.ix_(perm, perm)]           # plane order -> subset order
            # boundary: local center row 127 from the raw z rows
            ze = zedges[core].astype(np.float64)   # [2, 4, 512]
            zlo = ze[0].copy()                     # z_p(center 127, cl-1)
            # SA_p[cl] = z_p[row+1, cl+dx'], dx' = (+1, 0, -1, -1); p3 row+0
            zhi = np.zeros((4, 512))
            zhi[0, 1:511] = ze[1][0, 2:512]
            zhi[1, 1:511] = ze[1][1, 1:511]
            zhi[2, 1:511] = ze[1][2, 0:510]
            zhi[3, 1:511] = ze[0][3, 0:510]        # p=3: same row, col-1
            mlo = _monomials16(zlo)
            mlo[1:, 0] = 0.0
            mlo[1:, 511] = 0.0                     # edge columns invalid
            mhi = _monomials16(zhi)
            g16 += mlo @ mhi.T
        g16 *= hi_sign[None, :]                    # SA planes hold -z_{p+4}
        g16[0, 0] = float(HP * WP)                 # ones*ones: exact pixel count
        hmat = 2.0 ** -8 * (w16 @ g16 @ w16.T)     # [klo, khi]
        out[b] = hmat.T.reshape(256)               # k = klo + 16*khi
    return out.astype(np.float32).reshape(4, 256, 1, 1)


def kernel(x, diff_hardness, output_hardness):
    global last_results
    from concourse.bass_utils import run_bass_kernel_spmd

    x = np.asarray(x, np.float32)
    dh = float(np.asarray(diff_hardness))
    oh = float(np.asarray(output_hardness))

    key = (dh, oh)
    if key not in _PROGRAM_CACHE:
        _PROGRAM_CACHE[key] = _build_program(dh, oh)
    nc = _PROGRAM_CACHE[key]

    in_maps = []
    for core in range(8):
        b, half = divmod(core, 2)
        r0 = 0 if half == 0 else 255
        in_maps.append({"xs": np.ascontiguousarray(x[b, 0, r0:r0 + NROWS_SLICE, :])})

    trace = bool(int(os.environ.get("KERNEL_TRACE", "0")))
    res = run_bass_kernel_spmd(nc, in_maps, core_ids=list(range(8)), trace=trace)
    last_results = res
    grams = [res.results[c]["gram"] for c in range(8)]
    zedges = [res.results[c]["zedge"] for c in range(8)]
    return _postprocess(grams, zedges)



# revision 4
# speedup vs baseline: 1.3165x; 1.3165x over previous
"""DiffLBP soft-histogram kernel for Trainium2 (8 NeuronCores).

Math: the per-pixel softmax over 256 LBP patterns factorizes exactly into a
product of 8 independent Bernoullis with q_p = 1/2 (1 + z_p),
z_p = tanh((oh/2)*tanh(dh*d_p)).  The histogram is a 16x16 Gram matrix of
z-monomials (4 low bits x 4 high bits) pushed through a constant Walsh +-1
transform (host).  Antipodal offsets give z_{p+4}(r,c) = -z_p((r,c)+off), so
only 4 z planes are computed; the "hi" side uses row+1 shifted copies routed
through a DRAM bounce (contiguous whole-row loads; the per-plane column
shifts are applied on-chip by DVE copies, which are free in the free dim).

Device program per core (one batch b, one 255-row half; tile 1 first):
  z phase (x2 tiles of 128 rows): combined DMA loads XA/XB row-windows;
    DVE computes the 4 diffs; ScalarE does two tanh passes -> z (bf16);
    z is bounced to DRAM (zd) with one contiguous 4KB/partition DMA.
  mono phase (x2): mlo tile (chunk-major [128, 64 chunks, 16 planes x 8
    cols]) gets the z singles + 11 composite monomial planes (5 batched DVE
    multiplies); mhi tile (plane-major) gets row-shifted singles via one
    contiguous zd load + col-shifted DVE copies (plane 3 straight from the
    z tile in SBUF), then its own composites.
  gram phase (x2): 64 matmuls accumulate into PSUM.  Tile 0 runs K=128
    (the straddle row center 128 is included on-device: its mhi row comes
    from zd row 128 = z-tile-1 row 0), tile 1 runs K=127 (its row 127 is
    the neighbouring core's center).  No host boundary fix needed.
"""

import os
import numpy as np
from contextlib import ExitStack

H = W = 512
HP = WP = 510          # valid center rows/cols
NROWS_SLICE = 257      # input rows per core slice

# plane slot -> subset bitmask of {z0,z1,z2,z3}; chosen so the 11 composite
# planes are produced by 5 batched DVE multiplies (see _products)
PLANE_SUBSET = [0b0000,
                0b0001, 0b0010, 0b0100, 0b1000,   # 1..4:   z0 z1 z2 z3
                0b0011, 0b0110, 0b1100,           # 5..7:   {01} {12} {23}
                0b0101, 0b1010,                   # 8..9:   {02} {13}
                0b0111, 0b1110, 0b1111,           # 10..12: {012} {123} {0123}
                0b1011, 0b1101,                   # 13..14: {013} {023}
                0b1001]                           # 15:     {03}

_PROGRAM_CACHE = {}
last_results = None  # BassKernelResults of the most recent run (for test harness)


def _products(nc, m, R):
    """Emit the 11 composite monomial planes from singles (planes 1..4) of a
    16-plane view m[[part], 16, ...]; writes planes 5..15 on R partitions."""
    nc.vector.tensor_mul(m[:R, 5:8], m[:R, 1:4], m[:R, 2:5])        # 01 12 23
    nc.vector.tensor_mul(m[:R, 8:10], m[:R, 1:3], m[:R, 3:5])       # 02 13
    nc.vector.tensor_mul(m[:R, 10:13], m[:R, 5:8], m[:R, 3:6])      # 012 123 0123
    nc.vector.tensor_mul(m[:R, 13:15], m[:R, 9:7:-1], m[:R, 1:5:3]) # 013 023
    nc.vector.tensor_mul(m[:R, 15:16], m[:R, 1:2], m[:R, 4:5])      # 03


def _build_program(dh: float, oh: float):
    import concourse.bacc as bacc
    import concourse.tile as tile
    from concourse import mybir
    import concourse.bass as bass

    f32 = mybir.dt.float32
    bf16 = mybir.dt.bfloat16
    Tanh = mybir.ActivationFunctionType.Tanh

    nc = bacc.Bacc("TRN2", target_bir_lowering=False, debug=False)
    xs_t = nc.dram_tensor("xs", [NROWS_SLICE, W], f32, kind="ExternalInput")
    gram = nc.dram_tensor("gram", [128, 128], f32, kind="ExternalOutput").ap()

    with tile.TileContext(nc) as tc, ExitStack() as ctx:
        xpool = ctx.enter_context(tc.tile_pool(name="x", bufs=2))
        dpool = ctx.enter_context(tc.tile_pool(name="d", bufs=2))
        tpool = ctx.enter_context(tc.tile_pool(name="t", bufs=2))
        zpool = ctx.enter_context(tc.tile_pool(name="z", bufs=1))
        mpool = ctx.enter_context(tc.tile_pool(name="m", bufs=1))
        ppool = ctx.enter_context(
            tc.tile_pool(name="ps", bufs=1, space=bass.MemorySpace.PSUM))
        drpool = ctx.enter_context(
            tc.tile_pool(name="dr", bufs=1, space=bass.MemorySpace.DRAM))

        # trigger the tanh ACT table load immediately (overlaps the X DMAs)
        warm = mpool.tile([1, 8], f32, tag="warm")
        nc.vector.memset(warm[:, :], 0.0)
        nc.scalar.activation(warm[:, :], warm[:, :], Tanh)

        # DRAM bounce for the row-shifted hi singles (cross-partition shift)
        zd = drpool.tile([256, 4, W], bf16, tag="zd")
        ps = ppool.tile([128, 128], f32, tag="ps")
        zsrc, mlos, mhis = {}, {}, {}
        KS = {0: 128, 1: 127}   # gram contraction depth per tile

        def z_phase(i):
            r0 = 128 * i
            # one DMA: xt[:, j, :] = xs[r0 + j + rl, :] for j in (0, 1)
            xt = xpool.tile([128, 2, W], f32, tag="xt")
            src = bass.AP(xs_t, r0 * W, [[W, 128], [W, 2], [1, W]])
            nc.sync.dma_start(xt[:], src)
            xa, xb = xt[:, 0, :], xt[:, 1, :]

            d = dpool.tile([128, 4, W], f32, tag="d")
            nc.vector.memset(d[:, 0, 0:1], 0.0)
            nc.vector.memset(d[:, 2:4, 511:512], 0.0)
            # d_p[cl] = X_{dy}[cl+dx] - XB[cl]   (cl = x-col = center_col + 1)
            nc.vector.tensor_sub(d[:, 0, 1:512], xa[:, 0:511], xb[:, 1:512])   # (-1,-1)
            nc.vector.tensor_sub(d[:, 1, 0:512], xa[:, 0:512], xb[:, 0:512])   # (-1, 0)
            nc.vector.tensor_sub(d[:, 2, 0:511], xa[:, 1:512], xb[:, 0:511])   # (-1,+1)
            nc.vector.tensor_sub(d[:, 3, 0:511], xb[:, 1:512], xb[:, 0:511])   # ( 0,+1)

            t = tpool.tile([128, 4, W], f32, tag="t")
            nc.scalar.activation(t[:, :, :], d[:, :, :], Tanh, scale=float(dh))
            z = zpool.tile([128, 4, W], bf16, tag=f"zsrc{i}")
            nc.scalar.activation(z[:, :, :], t[:, :, :], Tanh, scale=float(oh) / 2.0)
            zsrc[i] = z
            nc.scalar.dma_start(zd[128 * i:128 * (i + 1), :, :], z[:, :, :])

        def mono_phase(i):
            z = zsrc[i]
            mlo = mpool.tile([128, 64, 128], bf16, tag=f"mlo{i}")
            ml = mlo[:].rearrange("k g (s c) -> k s g c", c=8)  # [128,16,64,8]
            mlos[i] = mlo
            mhi = mpool.tile([128, 16, W], bf16, tag=f"mhi{i}")
            mhis[i] = mhi
            zr = z[:].rearrange("k q (g c) -> k q g c", c=8)
            nc.vector.tensor_copy(ml[:, 1:5, :, :], zr[:, :, :, :])
            # lo side: ones + edge zeroing (singles already copied)
            nc.vector.memset(ml[:, 0, :, :], 1.0)
            nc.vector.memset(mlo[:, 0, 8:40:8], 0.0)    # cl=0   (g=0, c=0)
            nc.vector.memset(mlo[:, 63, 15:47:8], 0.0)  # cl=511 (g=63, c=7)
            _products(nc, ml, 128)

            # hi side: contiguous whole-row load from the bounce, then
            # col-shifted copies.  R rows of zd starting at 128i+1.
            R = KS[i]
            mraw = mpool.tile([128, 4, W], bf16, tag=f"mraw{i}")
            nc.sync.dma_start(mraw[0:R, :, :], zd[128 * i + 1:128 * i + 1 + R, :, :])
            # SA_p[rl, cl] = z_p[rl + dy'_p, cl + dx'_p], dx' = (+1, 0, -1, -1)
            nc.vector.tensor_copy(mhi[0:R, 1, 0:511], mraw[0:R, 0, 1:512])
            nc.vector.tensor_copy(mhi[0:R, 2, 0:512], mraw[0:R, 1, 0:512])
            nc.vector.tensor_copy(mhi[0:R, 3, 1:512], mraw[0:R, 2, 0:511])
            nc.vector.tensor_copy(mhi[0:128, 4, 1:512], z[0:128, 3, 0:511])
            nc.vector.memset(mhi[:, 0, :], 1.0)
            nc.vector.memset(mhi[:, 1:5, 0:512:511], 0.0)
            _products(nc, mhi, R)

        def gram_phase(i, first, last):
            K = KS[i]
            for g in range(64):
                nc.tensor.matmul(
                    ps[:, :],
                    mlos[i][0:K, g, :],
                    mhis[i][0:K, :, 8 * g:8 * g + 8],
                    start=(first and g == 0),
                    stop=(last and g == 63),
                )

        # tile 1 first: its zd rows (129..255) are self-contained, so its
        # gram unblocks early; tile 0's mhi needs zd row 128 (z-tile-1 row 0)
        z_phase(1)
        z_phase(0)
        mono_phase(1)
        gram_phase(1, True, False)
        mono_phase(0)
        gram_phase(0, False, True)

        gout = mpool.tile([128, 128], f32, tag="gout")
        nc.vector.tensor_copy(gout[:, :], ps[:, :])
        nc.sync.dma_start(gram[:, :], gout[:, :])

    nc.compile()
    return nc


def _walsh16():
    sgn = 2.0 * ((np.arange(16)[:, None] >> np.arange(4)[None, :]) & 1) - 1.0
    w = np.ones((16, 16))
    for k in range(16):
        for s in range(16):
            v = 1.0
            for p in range(4):
                if s >> p & 1:
                    v *= sgn[k, p]
            w[k, s] = v
    return w


def _postprocess(grams):
    """grams: 8x [128,128] f32 -> [4,256,1,1]."""
    perm = np.argsort(PLANE_SUBSET)  # subset-index -> plane-index
    w16 = _walsh16()
    hi_sign = np.array([(-1.0) ** bin(s).count("1") for s in range(16)])
    out = np.zeros((4, 256), np.float64)
    for b in range(4):
        g16 = np.zeros((16, 16))
        for half in range(2):
            gr = grams[2 * b + half].astype(np.float64).reshape(16, 8, 16, 8)
            g = np.einsum("sgtg->st", gr)          # sum the 8 diagonal blocks
            g16 += g[np.ix_(perm, perm)]           # plane order -> subset order
        g16 *= hi_sign[None, :]                    # SA planes hold -z_{p+4}
        g16[0, 0] = float(HP * WP)                 # ones*ones: exact pixel count
        hmat = 2.0 ** -8 * (w16 @ g16 @ w16.T)     # [klo, khi]
        out[b] = hmat.T.reshape(256)               # k = klo + 16*khi
    return out.astype(np.float32).reshape(4, 256, 1, 1)


def kernel(x, diff_hardness, output_hardness):
    global last_results
    from concourse.bass_utils import run_bass_kernel_spmd

    x = np.asarray(x, np.float32)
    dh = float(np.asarray(diff_hardness))
    oh = float(np.asarray(output_hardness))

    key = (dh, oh)
    if key not in _PROGRAM_CACHE:
        _PROGRAM_CACHE[key] = _build_program(dh, oh)
    nc = _PROGRAM_CACHE[key]

    in_maps = []
    for core in range(8):
        b, half = divmod(core, 2)
        r0 = 0 if half == 0 else 255
        in_maps.append({"xs": np.ascontiguousarray(x[b, 0, r0:r0 + NROWS_SLICE, :])})

    trace = bool(int(os.environ.get("KERNEL_TRACE", "0")))
    res = run_bass_kernel_spmd(nc, in_maps, core_ids=list(range(8)), trace=trace)
    last_results = res
    grams = [res.results[c]["gram"] for c in range(8)]
    return _postprocess(grams)


# revision 7
# speedup vs baseline: 1.4134x; 1.0736x over previous
"""DiffLBP soft-histogram kernel for Trainium2 (8 NeuronCores).

Math: the per-pixel softmax over 256 LBP patterns factorizes exactly into a
product of 8 independent Bernoullis with q_p = 1/2 (1 + z_p),
z_p = tanh((oh/2)*tanh(dh*d_p)).  The histogram is a 16x16 Gram matrix of
z-monomials (4 low bits x 4 high bits) pushed through a constant Walsh +-1
transform (host).  Antipodal offsets give z_{p+4}(r,c) = -z_p((r,c)+off), so
only 4 z planes are computed; the "hi" side needs row+1 shifted copies: the
cross-partition shift is done ON-CHIP by TensorE (matmul with a shifted
identity into PSUM); the per-plane column shifts are applied by the DVE
copies that drain PSUM (free in the free dim).

Device program per core (one batch b, one 255-row half; tile 1 first):
  z phase (x2 tiles of 128 rows): combined DMA loads XA/XB row-windows;
    DVE computes the 4 diffs; ScalarE does two tanh passes -> z (bf16).
  shift (x2): TensorE multiplies z planes 0..2 by a subdiagonal identity
    -> PSUM holds z[rl+1] on partition rl (tile 0 also accumulates a
    selector matmul that injects z-tile-1 row 0 into row 127).
  mono phase (x2): mlo tile (chunk-major [128, 64 chunks, 16 planes x 8
    cols]) gets the z singles (GPSIMD copy) + 11 composite monomial planes
    (5 batched DVE multiplies); mhi tile (plane-major) gets its singles by
    col-shifted DVE copies from the shift PSUM (plane 3 straight from the
    z tile via GPSIMD), then its own composites.
  gram phase (x2): 64 matmuls accumulate into PSUM.  Tile 0 runs K=128
    (the straddle row center 128 is included on-device), tile 1 K=127 (its
    row 127 is the neighbouring core's center).  No host boundary fix.
"""

import os
import numpy as np
from contextlib import ExitStack

H = W = 512
HP = WP = 510          # valid center rows/cols
NROWS_SLICE = 257      # input rows per core slice

# plane slot -> subset bitmask of {z0,z1,z2,z3}; chosen so the 11 composite
# planes are produced by 5 batched DVE multiplies (see _products)
PLANE_SUBSET = [0b0000,
                0b0001, 0b0010, 0b0100, 0b1000,   # 1..4:   z0 z1 z2 z3
                0b0011, 0b0110, 0b1100,           # 5..7:   {01} {12} {23}
                0b0101, 0b1010,                   # 8..9:   {02} {13}
                0b0111, 0b1110, 0b1111,           # 10..12: {012} {123} {0123}
                0b1011, 0b1101,                   # 13..14: {013} {023}
                0b1001]                           # 15:     {03}

_PROGRAM_CACHE = {}
last_results = None  # BassKernelResults of the most recent run (for test harness)


def _products(nc, m, R):
    """Emit the 11 composite monomial planes from singles (planes 1..4) of a
    16-plane view m[[part], 16, ...]; writes planes 5..15 on R partitions."""
    nc.vector.tensor_mul(m[:R, 5:8], m[:R, 1:4], m[:R, 2:5])        # 01 12 23
    nc.vector.tensor_mul(m[:R, 8:10], m[:R, 1:3], m[:R, 3:5])       # 02 13
    nc.vector.tensor_mul(m[:R, 10:13], m[:R, 5:8], m[:R, 3:6])      # 012 123 0123
    nc.vector.tensor_mul(m[:R, 13:15], m[:R, 9:7:-1], m[:R, 1:5:3]) # 013 023
    nc.vector.tensor_mul(m[:R, 15:16], m[:R, 1:2], m[:R, 4:5])      # 03


def _build_program(dh: float, oh: float):
    import concourse.bacc as bacc
    import concourse.tile as tile
    from concourse import mybir
    import concourse.bass as bass

    f32 = mybir.dt.float32
    bf16 = mybir.dt.bfloat16
    Tanh = mybir.ActivationFunctionType.Tanh

    nc = bacc.Bacc("TRN2", target_bir_lowering=False, debug=False)
    xs_t = nc.dram_tensor("xs", [NROWS_SLICE, W], f32, kind="ExternalInput")
    id_t = nc.dram_tensor("ident", [128, 256], bf16, kind="ExternalInput")
    gram = nc.dram_tensor("gram", [128, 128], f32, kind="ExternalOutput").ap()

    with tile.TileContext(nc) as tc, ExitStack() as ctx:
        xpool = ctx.enter_context(tc.tile_pool(name="x", bufs=2))
        dpool = ctx.enter_context(tc.tile_pool(name="d", bufs=2))
        tpool = ctx.enter_context(tc.tile_pool(name="t", bufs=2))
        zpool = ctx.enter_context(tc.tile_pool(name="z", bufs=1))
        mpool = ctx.enter_context(tc.tile_pool(name="m", bufs=1))
        ppool = ctx.enter_context(
            tc.tile_pool(name="ps", bufs=1, space=bass.MemorySpace.PSUM))

        # trigger the tanh ACT table load immediately (overlaps the X DMAs)
        warm = mpool.tile([1, 8], f32, tag="warm")
        nc.vector.memset(warm[:, :], 0.0)
        nc.scalar.activation(warm[:, :], warm[:, :], Tanh)

        # shifted-identity weights for the TensorE partition shift
        identt = mpool.tile([128, 256], bf16, tag="identt")
        nc.sync.dma_start(identt[:, :], id_t.ap())

        ps = ppool.tile([128, 128], f32, tag="ps")
        pshift = {i: ppool.tile([128, 3, W], f32, name=f"pshift{i}",
                                tag=f"pshift{i}") for i in (0, 1)}
        zsrc, mlos, mhis = {}, {}, {}
        KS = {0: 128, 1: 127}   # gram contraction depth per tile

        def z_phase(i):
            r0 = 128 * i
            # one DMA: xt[:, j, :] = xs[r0 + j + rl, :] for j in (0, 1)
            xt = xpool.tile([128, 2, W], f32, tag="xt")
            src = bass.AP(xs_t, r0 * W, [[W, 128], [W, 2], [1, W]])
            nc.sync.dma_start(xt[:], src)
            xa, xb = xt[:, 0, :], xt[:, 1, :]

            d = dpool.tile([128, 4, W], f32, tag="d")
            nc.vector.memset(d[:, 0, 0:1], 0.0)
            nc.vector.memset(d[:, 2:4, 511:512], 0.0)
            # d_p[cl] = X_{dy}[cl+dx] - XB[cl]   (cl = x-col = center_col + 1)
            nc.vector.tensor_sub(d[:, 0, 1:512], xa[:, 0:511], xb[:, 1:512])   # (-1,-1)
            nc.vector.tensor_sub(d[:, 1, 0:512], xa[:, 0:512], xb[:, 0:512])   # (-1, 0)
            nc.vector.tensor_sub(d[:, 2, 0:511], xa[:, 1:512], xb[:, 0:511])   # (-1,+1)
            nc.vector.tensor_sub(d[:, 3, 0:511], xb[:, 1:512], xb[:, 0:511])   # ( 0,+1)

            t = tpool.tile([128, 4, W], f32, tag="t")
            nc.scalar.activation(t[:, :, :], d[:, :, :], Tanh, scale=float(dh))
            z = zpool.tile([128, 4, W], bf16, tag=f"zsrc{i}")
            nc.scalar.activation(z[:, :, :], t[:, :, :], Tanh, scale=float(oh) / 2.0)
            zsrc[i] = z

        def shift_phase(i):
            # pshift[i][rl, p, :] = z_p(row rl+1) for p in 0..2: TensorE
            # subdiagonal-identity matmul; tile 0 row 127 = z-tile-1 row 0
            # via the selector in identt cols 128:256.
            for p in range(3):
                nc.tensor.matmul(pshift[i][:, p, :], identt[:, 0:128],
                                 zsrc[i][:, p, :], start=True, stop=(i == 1))
            if i == 0:
                for p in range(3):
                    nc.tensor.matmul(pshift[0][:, p, :], identt[:, 128:256],
                                     zsrc[1][:, p, :], start=False, stop=True)

        def mono_phase(i):
            z = zsrc[i]
            mlo = mpool.tile([128, 64, 128], bf16, tag=f"mlo{i}")
            ml = mlo[:].rearrange("k g (s c) -> k s g c", c=8)  # [128,16,64,8]
            mlos[i] = mlo
            mhi = mpool.tile([128, 16, W], bf16, tag=f"mhi{i}")
            mhis[i] = mhi
            zr = z[:].rearrange("k q (g c) -> k q g c", c=8)
            nc.gpsimd.tensor_copy(ml[:, 1:5, :, :], zr[:, :, :, :])
            # lo side: ones + edge zeroing (singles already copied)
            nc.gpsimd.memset(ml[:, 0, :, :], 1.0)
            nc.vector.memset(mlo[:, 0, 8:40:8], 0.0)    # cl=0   (g=0, c=0)
            nc.vector.memset(mlo[:, 63, 15:47:8], 0.0)  # cl=511 (g=63, c=7)
            _products(nc, ml, 128)

            # hi side: drain the TensorE shift PSUM with col-shifted copies.
            # SA_p[rl, cl] = z_p[rl + dy'_p, cl + dx'_p], dx' = (+1, 0, -1, -1)
            R = KS[i]
            psh = pshift[i]
            nc.vector.tensor_copy(mhi[0:R, 1, 0:511], psh[0:R, 0, 1:512])
            nc.vector.tensor_copy(mhi[0:R, 2, 0:512], psh[0:R, 1, 0:512])
            nc.vector.tensor_copy(mhi[0:R, 3, 1:512], psh[0:R, 2, 0:511])
            nc.gpsimd.tensor_copy(mhi[0:128, 4, 1:512], z[0:128, 3, 0:511])
            nc.gpsimd.memset(mhi[:, 0, :], 1.0)
            nc.vector.memset(mhi[:, 1:5, 0:512:511], 0.0)
            _products(nc, mhi, R)

        def gram_phase(i, first, last):
            K = KS[i]
            for g in range(64):
                nc.tensor.matmul(
                    ps[:, :],
                    mlos[i][0:K, g, :],
                    mhis[i][0:K, :, 8 * g:8 * g + 8],
                    start=(first and g == 0),
                    stop=(last and g == 63),
                )

        # tile 1 first: its shift is self-contained, so its gram unblocks
        # early; tile 0's shift needs z-tile-1 row 0 (the selector matmul)
        z_phase(1)
        z_phase(0)
        shift_phase(1)
        shift_phase(0)
        mono_phase(1)
        gram_phase(1, True, False)
        mono_phase(0)
        gram_phase(0, False, True)

        gout = mpool.tile([128, 128], f32, tag="gout")
        nc.vector.tensor_copy(gout[:, :], ps[:, :])
        nc.sync.dma_start(gram[:, :], gout[:, :])

    nc.compile()
    return nc


def _walsh16():
    sgn = 2.0 * ((np.arange(16)[:, None] >> np.arange(4)[None, :]) & 1) - 1.0
    w = np.ones((16, 16))
    for k in range(16):
        for s in range(16):
            v = 1.0
            for p in range(4):
                if s >> p & 1:
                    v *= sgn[k, p]
            w[k, s] = v
    return w


def _postprocess(grams):
    """grams: 8x [128,128] f32 -> [4,256,1,1]."""
    perm = np.argsort(PLANE_SUBSET)  # subset-index -> plane-index
    w16 = _walsh16()
    hi_sign = np.array([(-1.0) ** bin(s).count("1") for s in range(16)])
    out = np.zeros((4, 256), np.float64)
    for b in range(4):
        g16 = np.zeros((16, 16))
        for half in range(2):
            gr = grams[2 * b + half].astype(np.float64).reshape(16, 8, 16, 8)
            g = np.einsum("sgtg->st", gr)          # sum the 8 diagonal blocks
            g16 += g[np.ix_(perm, perm)]           # plane order -> subset order
        g16 *= hi_sign[None, :]                    # SA planes hold -z_{p+4}
        g16[0, 0] = float(HP * WP)                 # ones*ones: exact pixel count
        hmat = 2.0 ** -8 * (w16 @ g16 @ w16.T)     # [klo, khi]
        out[b] = hmat.T.reshape(256)               # k = klo + 16*khi
    return out.astype(np.float32).reshape(4, 256, 1, 1)


def _ident_np():
    import ml_dtypes
    a = np.zeros((128, 256), dtype=np.float32)
    for m in range(127):
        a[m + 1, m] = 1.0          # subdiagonal: out[m] = z[m+1]
    a[0, 128 + 127] = 1.0          # selector: out[127] = other-tile z[0]
    return a.astype(ml_dtypes.bfloat16)


def kernel(x, diff_hardness, output_hardness):
    global last_results
    from concourse.bass_utils import run_bass_kernel_spmd

    x = np.asarray(x, np.float32)
    dh = float(np.asarray(diff_hardness))
    oh = float(np.asarray(output_hardness))

    key = (dh, oh)
    if key not in _PROGRAM_CACHE:
        _PROGRAM_CACHE[key] = _build_program(dh, oh)
    nc = _PROGRAM_CACHE[key]

    ident = _ident_np()
    in_maps = []
    for core in range(8):
        b, half = divmod(core, 2)
        r0 = 0 if half == 0 else 255
        in_maps.append({
            "xs": np.ascontiguousarray(x[b, 0, r0:r0 + NROWS_SLICE, :]),
            "ident": ident,
        })

    trace = bool(int(os.environ.get("KERNEL_TRACE", "0")))
    res = run_bass_kernel_spmd(nc, in_maps, core_ids=list(range(8)), trace=trace)
    last_results = res
    grams = [res.results[c]["gram"] for c in range(8)]
    return _postprocess(grams)


# revision 8
# speedup vs baseline: 1.6903x; 1.1959x over previous
"""DiffLBP soft-histogram kernel for Trainium2 (8 NeuronCores).

Math: the per-pixel softmax over 256 LBP patterns factorizes exactly into a
product of 8 independent Bernoullis with q_p = 1/2 (1 + z_p),
z_p = tanh((oh/2)*tanh(dh*d_p)).  The histogram is a 16x16 Gram matrix of
z-monomials (4 low bits x 4 high bits) pushed through a constant Walsh +-1
transform (host).  Antipodal offsets give z_{p+4}(r,c) = -z_p((r,c)+off), so
only 4 z planes are computed; the "hi" side needs row+1 shifted copies: the
cross-partition shift is done ON-CHIP by TensorE (matmul with a shifted
identity into PSUM); the per-plane column shifts are applied by the DVE
copies that drain PSUM (free in the free dim).

Device program per core (one batch b, one 255-row half; tile 1 first):
  z phase (x2 tiles of 128 rows): combined DMA loads XA/XB row-windows;
    DVE computes the 4 diffs; ScalarE does two tanh passes -> z (bf16).
  shift (x2): TensorE multiplies z planes 0..2 by a subdiagonal identity
    -> PSUM holds z[rl+1] on partition rl (tile 0 also accumulates a
    selector matmul that injects z-tile-1 row 0 into row 127).
  mono phase (x2): mlo tile (chunk-major [128, 64 chunks, 16 planes x 8
    cols]) gets the z singles (GPSIMD copy) + 11 composite monomial planes
    (5 batched DVE multiplies); mhi tile (plane-major) gets its singles by
    col-shifted DVE copies from the shift PSUM (plane 3 straight from the
    z tile via GPSIMD), then its own composites.
  gram phase (x2): 64 matmuls accumulate into PSUM.  Tile 0 runs K=128
    (the straddle row center 128 is included on-device), tile 1 K=127 (its
    row 127 is the neighbouring core's center).  No host boundary fix.
"""

import os
import numpy as np
from contextlib import ExitStack

H = W = 512
HP = WP = 510          # valid center rows/cols
NROWS_SLICE = 257      # input rows per core slice

# plane slot -> subset bitmask of {z0,z1,z2,z3}; chosen so the 11 composite
# planes are produced by 5 batched DVE multiplies (see _products)
PLANE_SUBSET = [0b0000,
                0b0001, 0b0010, 0b0100, 0b1000,   # 1..4:   z0 z1 z2 z3
                0b0011, 0b0110, 0b1100,           # 5..7:   {01} {12} {23}
                0b0101, 0b1010,                   # 8..9:   {02} {13}
                0b0111, 0b1110, 0b1111,           # 10..12: {012} {123} {0123}
                0b1011, 0b1101,                   # 13..14: {013} {023}
                0b1001]                           # 15:     {03}

_PROGRAM_CACHE = {}
last_results = None  # BassKernelResults of the most recent run (for test harness)


def _products(nc, m, R):
    """Emit the 11 composite monomial planes from singles (planes 1..4) of a
    16-plane view m[[part], 16, ...]; writes planes 5..15 on R partitions."""
    nc.vector.tensor_mul(m[:R, 5:8], m[:R, 1:4], m[:R, 2:5])        # 01 12 23
    nc.vector.tensor_mul(m[:R, 8:10], m[:R, 1:3], m[:R, 3:5])       # 02 13
    nc.vector.tensor_mul(m[:R, 10:13], m[:R, 5:8], m[:R, 3:6])      # 012 123 0123
    nc.vector.tensor_mul(m[:R, 13:15], m[:R, 9:7:-1], m[:R, 1:5:3]) # 013 023
    nc.vector.tensor_mul(m[:R, 15:16], m[:R, 1:2], m[:R, 4:5])      # 03


def _build_program(dh: float, oh: float):
    import concourse.bacc as bacc
    import concourse.tile as tile
    from concourse import mybir
    import concourse.bass as bass

    f32 = mybir.dt.float32
    bf16 = mybir.dt.bfloat16
    Tanh = mybir.ActivationFunctionType.Tanh

    nc = bacc.Bacc("TRN2", target_bir_lowering=False, debug=False)
    xs_t = nc.dram_tensor("xs", [NROWS_SLICE, W], f32, kind="ExternalInput")
    id_t = nc.dram_tensor("ident", [128, 256], bf16, kind="ExternalInput")
    gram = nc.dram_tensor("gram", [128, 128], f32, kind="ExternalOutput").ap()

    with tile.TileContext(nc) as tc, ExitStack() as ctx:
        xpool = ctx.enter_context(tc.tile_pool(name="x", bufs=2))
        dpool = ctx.enter_context(tc.tile_pool(name="d", bufs=2))
        tpool = ctx.enter_context(tc.tile_pool(name="t", bufs=2))
        zpool = ctx.enter_context(tc.tile_pool(name="z", bufs=1))
        mpool = ctx.enter_context(tc.tile_pool(name="m", bufs=1))
        ppool = ctx.enter_context(
            tc.tile_pool(name="ps", bufs=1, space=bass.MemorySpace.PSUM))

        # trigger the tanh ACT table load immediately (overlaps the X DMAs)
        warm = mpool.tile([1, 8], f32, tag="warm")
        nc.vector.memset(warm[:, :], 0.0)
        nc.scalar.activation(warm[:, :], warm[:, :], Tanh)

        # shifted-identity weights for the TensorE partition shift
        identt = mpool.tile([128, 256], bf16, tag="identt")
        nc.sync.dma_start(identt[:, :], id_t.ap())

        ps = ppool.tile([128, 128], f32, tag="ps")
        pshift = {i: ppool.tile([128, 3, W], f32, name=f"pshift{i}",
                                tag=f"pshift{i}") for i in (0, 1)}
        zsrc, mlos, mhis = {}, {}, {}
        KS = {0: 128, 1: 127}   # gram contraction depth per tile

        def z_phase(i):
            r0 = 128 * i
            # one DMA: xt[:, j, :] = xs[r0 + j + rl, :] for j in (0, 1)
            xt = xpool.tile([128, 2, W], f32, tag="xt")
            src = bass.AP(xs_t, r0 * W, [[W, 128], [W, 2], [1, W]])
            nc.sync.dma_start(xt[:], src)
            xa, xb = xt[:, 0, :], xt[:, 1, :]

            d = dpool.tile([128, 4, W], f32, tag="d")
            nc.vector.memset(d[:, 0, 0:1], 0.0)
            nc.vector.memset(d[:, 2:4, 511:512], 0.0)
            # d_p[cl] = X_{dy}[cl+dx] - XB[cl]   (cl = x-col = center_col + 1)
            nc.vector.tensor_sub(d[:, 0, 1:512], xa[:, 0:511], xb[:, 1:512])   # (-1,-1)
            nc.vector.tensor_sub(d[:, 1, 0:512], xa[:, 0:512], xb[:, 0:512])   # (-1, 0)
            nc.vector.tensor_sub(d[:, 2, 0:511], xa[:, 1:512], xb[:, 0:511])   # (-1,+1)
            nc.vector.tensor_sub(d[:, 3, 0:511], xb[:, 1:512], xb[:, 0:511])   # ( 0,+1)

            t = tpool.tile([128, 4, W], f32, tag="t")
            nc.scalar.activation(t[:, :, :], d[:, :, :], Tanh, scale=float(dh))
            z = zpool.tile([128, 4, W], bf16, tag=f"zsrc{i}")
            nc.scalar.activation(z[:, :, :], t[:, :, :], Tanh, scale=float(oh) / 2.0)
            zsrc[i] = z

        def shift_phase(i):
            # pshift[i][rl, p, :] = z_p(row rl+1) for p in 0..2: TensorE
            # subdiagonal-identity matmul; tile 0 row 127 = z-tile-1 row 0
            # via the selector in identt cols 128:256.
            for p in range(3):
                nc.tensor.matmul(pshift[i][:, p, :], identt[:, 0:128],
                                 zsrc[i][:, p, :], start=True, stop=(i == 1))
            if i == 0:
                for p in range(3):
                    nc.tensor.matmul(pshift[0][:, p, :], identt[:, 128:256],
                                     zsrc[1][:, p, :], start=False, stop=True)

        def mono_phase(i):
            z = zsrc[i]
            mlo = mpool.tile([128, 64, 128], bf16, tag=f"mlo{i}")
            ml = mlo[:].rearrange("k g (s c) -> k s g c", c=8)  # [128,16,64,8]
            mlos[i] = mlo
            mhi = mpool.tile([128, 16, W], bf16, tag=f"mhi{i}")
            mhis[i] = mhi
            zr = z[:].rearrange("k q (g c) -> k q g c", c=8)
            nc.vector.tensor_copy(ml[:, 1:5, :, :], zr[:, :, :, :])
            # lo side: ones + edge zeroing (singles already copied)
            nc.gpsimd.memset(ml[:, 0, :, :], 1.0)
            nc.vector.memset(mlo[:, 0, 8:40:8], 0.0)    # cl=0   (g=0, c=0)
            nc.vector.memset(mlo[:, 63, 15:47:8], 0.0)  # cl=511 (g=63, c=7)
            _products(nc, ml, 128)

            # hi side: drain the TensorE shift PSUM with col-shifted copies.
            # SA_p[rl, cl] = z_p[rl + dy'_p, cl + dx'_p], dx' = (+1, 0, -1, -1)
            R = KS[i]
            psh = pshift[i]
            Copy = __import__("concourse.mybir", fromlist=["x"]).ActivationFunctionType.Copy
            nc.scalar.activation(mhi[0:R, 1, 0:511], psh[0:R, 0, 1:512], Copy)
            nc.scalar.activation(mhi[0:R, 2, 0:512], psh[0:R, 1, 0:512], Copy)
            nc.scalar.activation(mhi[0:R, 3, 1:512], psh[0:R, 2, 0:511], Copy)
            nc.vector.tensor_copy(mhi[0:128, 4, 1:512], z[0:128, 3, 0:511])
            nc.gpsimd.memset(mhi[:, 0, :], 1.0)
            nc.vector.memset(mhi[:, 1:5, 0:512:511], 0.0)
            _products(nc, mhi, R)

        def gram_phase(i, first, last):
            K = KS[i]
            for g in range(64):
                nc.tensor.matmul(
                    ps[:, :],
                    mlos[i][0:K, g, :],
                    mhis[i][0:K, :, 8 * g:8 * g + 8],
                    start=(first and g == 0),
                    stop=(last and g == 63),
                )

        # tile 1 first: its shift is self-contained, so its gram unblocks
        # early; tile 0's shift needs z-tile-1 row 0 (the selector matmul)
        z_phase(1)
        z_phase(0)
        shift_phase(1)
        shift_phase(0)
        mono_phase(1)
        gram_phase(1, True, False)
        mono_phase(0)
        gram_phase(0, False, True)

        gout = mpool.tile([128, 128], f32, tag="gout")
        nc.vector.tensor_copy(gout[:, :], ps[:, :])
        nc.sync.dma_start(gram[:, :], gout[:, :])

    nc.compile()
    return nc


def _walsh16():
    sgn = 2.0 * ((np.arange(16)[:, None] >> np.arange(4)[None, :]) & 1) - 1.0
    w = np.ones((16, 16))
    for k in range(16):
        for s in range(16):
            v = 1.0
            for p in range(4):
                if s >> p & 1:
                    v *= sgn[k, p]
            w[k, s] = v
    return w


def _postprocess(grams):
    """grams: 8x [128,128] f32 -> [4,256,1,1]."""
    perm = np.argsort(PLANE_SUBSET)  # subset-index -> plane-index
    w16 = _walsh16()
    hi_sign = np.array([(-1.0) ** bin(s).count("1") for s in range(16)])
    out = np.zeros((4, 256), np.float64)
    for b in range(4):
        g16 = np.zeros((16, 16))
        for half in range(2):
            gr = grams[2 * b + half].astype(np.float64).reshape(16, 8, 16, 8)
            g = np.einsum("sgtg->st", gr)          # sum the 8 diagonal blocks
            g16 += g[np.ix_(perm, perm)]           # plane order -> subset order
        g16 *= hi_sign[None, :]                    # SA planes hold -z_{p+4}
        g16[0, 0] = float(HP * WP)                 # ones*ones: exact pixel count
        hmat = 2.0 ** -8 * (w16 @ g16 @ w16.T)     # [klo, khi]
        out[b] = hmat.T.reshape(256)               # k = klo + 16*khi
    return out.astype(np.float32).reshape(4, 256, 1, 1)


def _ident_np():
    import ml_dtypes
    a = np.zeros((128, 256), dtype=np.float32)
    for m in range(127):
        a[m + 1, m] = 1.0          # subdiagonal: out[m] = z[m+1]
    a[0, 128 + 127] = 1.0          # selector: out[127] = other-tile z[0]
    return a.astype(ml_dtypes.bfloat16)


def kernel(x, diff_hardness, output_hardness):
    global last_results
    from concourse.bass_utils import run_bass_kernel_spmd

    x = np.asarray(x, np.float32)
    dh = float(np.asarray(diff_hardness))
    oh = float(np.asarray(output_hardness))

    key = (dh, oh)
    if key not in _PROGRAM_CACHE:
        _PROGRAM_CACHE[key] = _build_program(dh, oh)
    nc = _PROGRAM_CACHE[key]

    ident = _ident_np()
    in_maps = []
    for core in range(8):
        b, half = divmod(core, 2)
        r0 = 0 if half == 0 else 255
        in_maps.append({
            "xs": np.ascontiguousarray(x[b, 0, r0:r0 + NROWS_SLICE, :]),
            "ident": ident,
        })

    trace = bool(int(os.environ.get("KERNEL_TRACE", "0")))
    res = run_bass_kernel_spmd(nc, in_maps, core_ids=list(range(8)), trace=trace)
    last_results = res
    grams = [res.results[c]["gram"] for c in range(8)]
    return _postprocess(grams)


# revision 10
# speedup vs baseline: 1.8509x; 1.0951x over previous
"""DiffLBP soft-histogram kernel for Trainium2 (8 NeuronCores).

Math: the per-pixel softmax over 256 LBP patterns factorizes exactly into a
product of 8 independent Bernoullis with q_p = 1/2 (1 + z_p),
z_p = tanh((oh/2)*tanh(dh*d_p)).  The histogram is a 16x16 Gram matrix of
z-monomials (4 low bits x 4 high bits) pushed through a constant Walsh +-1
transform (host).  Antipodal offsets give z_{p+4}(r,c) = -z_p((r,c)+off), so
only 4 z planes are computed; the "hi" side needs (row+1, col+dx) shifted
copies: both shifts are done by TensorE (subdiagonal-identity matmul with
col-offset APs into PSUM), drained to SBUF by one ScalarE copy per tile.

Device program per core (one batch b, one 255-row half; tile 1 first):
  z phase (x2 tiles of 128 rows): SWDGE DMA loads XA/XB row-windows cast to
    bf16; DVE computes the 4 diffs; ScalarE does two tanh passes, writing
    the z singles straight into the plane-major mlo tile (planes 1..4).
  shift (x2): TensorE multiplies z planes 0..2 by a subdiagonal identity
    with per-plane column offsets -> PSUM holds the row+col shifted hi
    singles (tile 0 also accumulates a selector matmul that injects
    z-tile-1 row 0 into row 127); ScalarE drains PSUM -> mhi planes 1..3.
  mono phase (x2): both mlo and mhi are plane-major [128, 16, 512]; the 11
    composite monomial planes are built by 4 batched DVE multiplies + 1
    GPSIMD multiply ({03}, which needs only singles so it runs early).
  gram phase (x2): 64 matmuls accumulate into PSUM (strided lhsT selects
    16 planes x 8 cols).  A zero-weight matmul warmup stream keeps the PE
    HAM un-throttled before the gram bursts.  Tile 0 runs K=128 (the
    straddle row center 128 is included on-device), tile 1 K=127 (its row
    127 is the neighbouring core's center).  No host boundary fix.
"""

import os
import numpy as np
from contextlib import ExitStack

H = W = 512
HP = WP = 510          # valid center rows/cols
NROWS_SLICE = 257      # input rows per core slice

# plane slot -> subset bitmask of {z0,z1,z2,z3}; chosen so the 11 composite
# planes are produced by 5 batched multiplies (see _products)
PLANE_SUBSET = [0b0000,
                0b0001, 0b0010, 0b0100, 0b1000,   # 1..4:   z0 z1 z2 z3
                0b0011, 0b0110, 0b1100,           # 5..7:   {01} {12} {23}
                0b0101, 0b1010,                   # 8..9:   {02} {13}
                0b0111, 0b1110, 0b1111,           # 10..12: {012} {123} {0123}
                0b1011, 0b1101,                   # 13..14: {013} {023}
                0b1001]                           # 15:     {03}

_PROGRAM_CACHE = {}
last_results = None  # BassKernelResults of the most recent run (for test harness)


def _products(nc, m, R):
    """Emit the 11 composite monomial planes from singles (planes 1..4) of a
    plane-major view m[[part], 16, W]; writes planes 5..15 on R partitions.
    The {03} plane needs only singles, so it goes to GPSIMD early."""
    nc.gpsimd.tensor_mul(m[:R, 15:16], m[:R, 1:2], m[:R, 4:5])      # 03
    nc.vector.tensor_mul(m[:R, 5:8], m[:R, 1:4], m[:R, 2:5])        # 01 12 23
    nc.vector.tensor_mul(m[:R, 8:10], m[:R, 1:3], m[:R, 3:5])       # 02 13
    nc.vector.tensor_mul(m[:R, 10:13], m[:R, 5:8], m[:R, 3:6])      # 012 123 0123
    nc.vector.tensor_mul(m[:R, 13:15], m[:R, 9:7:-1], m[:R, 1:5:3]) # 013 023


def _build_program(dh: float, oh: float):
    import concourse.bacc as bacc
    import concourse.tile as tile
    from concourse import mybir
    import concourse.bass as bass

    f32 = mybir.dt.float32
    bf16 = mybir.dt.bfloat16
    Tanh = mybir.ActivationFunctionType.Tanh
    Copy = mybir.ActivationFunctionType.Copy

    nc = bacc.Bacc("TRN2", target_bir_lowering=False, debug=False)
    xs_t = nc.dram_tensor("xs", [NROWS_SLICE, W], f32, kind="ExternalInput")
    id_t = nc.dram_tensor("ident", [128, 256], bf16, kind="ExternalInput")
    gram = nc.dram_tensor("gram", [128, 128], f32, kind="ExternalOutput").ap()

    with tile.TileContext(nc) as tc, ExitStack() as ctx:
        xpool = ctx.enter_context(tc.tile_pool(name="x", bufs=2))
        dpool = ctx.enter_context(tc.tile_pool(name="d", bufs=2))
        tpool = ctx.enter_context(tc.tile_pool(name="t", bufs=2))
        mpool = ctx.enter_context(tc.tile_pool(name="m", bufs=1))
        ppool = ctx.enter_context(
            tc.tile_pool(name="ps", bufs=1, space=bass.MemorySpace.PSUM))

        # x loads first (they gate everything); SWDGE casts f32 -> bf16
        xts = {}
        for i in (1, 0):
            xt = xpool.tile([128, 2, W], bf16, name=f"xt{i}", tag=f"xt{i}")
            src = bass.AP(xs_t, 128 * i * W, [[W, 128], [W, 2], [1, W]])
            nc.gpsimd.dma_start(xt[:], src)
            xts[i] = xt

        # shifted-identity weights for the TensorE partition shift
        identt = mpool.tile([128, 256], bf16, tag="identt")
        nc.sync.dma_start(identt[:, :], id_t.ap())

        # trigger the tanh ACT table load immediately (overlaps the X DMAs)
        warm = mpool.tile([1, 8], f32, tag="warm")
        nc.vector.memset(warm[:, :], 0.0)
        nc.scalar.activation(warm[:, :], warm[:, :], Tanh)

        # zero stationary for the PE HAM warmup (contributes 0 to the gram)
        wz = mpool.tile([128, 128], bf16, tag="wz")
        nc.vector.memset(wz[:, :], 0.0)

        ps = ppool.tile([128, 128], f32, tag="ps")
        pshift = {i: ppool.tile([128, 3, W], f32, name=f"pshift{i}",
                                tag=f"pshift{i}") for i in (0, 1)}
        mlos, mhis = {}, {}
        KS = {0: 128, 1: 127}   # gram contraction depth per tile

        # mlo: plane-major (the gram's MOVING operand tolerates 2 free
        # dims); z singles live in planes 1..4.  mhi: chunk-major [128, 64
        # chunks, 16 planes x 8 cols] (the STATIONARY needs 1 free dim).
        mhv = {}
        for i in (1, 0):
            mlos[i] = mpool.tile([128, 16, W], bf16, name=f"mlo{i}", tag=f"mlo{i}")
            mhis[i] = mpool.tile([128, 64, 128], bf16, name=f"mhi{i}", tag=f"mhi{i}")
            mhv[i] = mhis[i][:].rearrange("k g (s c) -> k s g c", c=8)
            nc.gpsimd.memset(mlos[i][:, 0, :], 1.0)
            nc.gpsimd.memset(mhis[i][:, :, 0:8], 1.0)

        def z_phase(i):
            xt = xts[i]
            xa, xb = xt[:, 0, :], xt[:, 1, :]
            d = dpool.tile([128, 4, W], bf16, name=f"d{i}", tag=f"d{i}")
            # d_p[cl] = X_{dy}[cl+dx] - XB[cl]   (cl = x-col = center_col + 1)
            nc.vector.tensor_sub(d[:, 0, 1:512], xa[:, 0:511], xb[:, 1:512])   # (-1,-1)
            nc.vector.tensor_sub(d[:, 1, 0:512], xa[:, 0:512], xb[:, 0:512])   # (-1, 0)
            nc.vector.tensor_sub(d[:, 2, 0:511], xa[:, 1:512], xb[:, 0:511])   # (-1,+1)
            nc.vector.tensor_sub(d[:, 3, 0:511], xb[:, 1:512], xb[:, 0:511])   # ( 0,+1)

            t = tpool.tile([128, 4, W], f32, name=f"t{i}", tag=f"t{i}")
            nc.scalar.activation(t[:, :, :], d[:, :, :], Tanh, scale=float(dh))
            # z singles straight into the plane-major mlo tile
            nc.scalar.activation(mlos[i][:, 1:5, :], t[:, :, :], Tanh,
                                 scale=float(oh) / 2.0)
            # invalid edge columns: zero ALL singles at cl=0 and cl=511
            # (before products, so composites inherit the zeros)
            nc.vector.memset(mlos[i][:, 1:5, 0:512:511], 0.0)

        # col windows for the shift matmuls: plane p shifts cols by
        # dx'_p = (+1, 0, -1); out col c reads z col c+dx'
        SHIFT_WIN = [  # (out_lo, out_hi, in_lo, in_hi)
            (0, 511, 1, 512),
            (0, 512, 0, 512),
            (1, 512, 0, 511),
        ]

        def shift_phase(i):
            # pshift[i][rl, p, c] = z_p(row rl+1, col c+dx') via TensorE;
            # tile 0 row 127 = z-tile-1 row 0 via the selector weights.
            for p, (ol, ohi, il, ihi) in enumerate(SHIFT_WIN):
                nc.tensor.matmul(pshift[i][:, p, ol:ohi], identt[:, 0:128],
                                 mlos[i][:, 1 + p, il:ihi],
                                 start=True, stop=(i == 1))
            if i == 0:
                for p, (ol, ohi, il, ihi) in enumerate(SHIFT_WIN):
                    nc.tensor.matmul(pshift[0][:, p, ol:ohi], identt[:, 128:256],
                                     mlos[1][:, 1 + p, il:ihi],
                                     start=False, stop=True)

        def mono_phase(i):
            mlo, mh = mlos[i], mhv[i]
            R = KS[i]
            _products(nc, mlo[:], 128)
            # hi singles: one ScalarE drain of the shift PSUM (planes 1..3,
            # already row+col shifted) into the chunk-major slots; plane 4 =
            # z_3 col-shifted by -1 from mlo plane 4 (2 copies: mod-8 split)
            psr = pshift[i][0:R, :, :].rearrange("k p (g c) -> k p g c", c=8)
            nc.scalar.activation(mh[0:R, 1:4, :, :], psr, Copy)
            nc.vector.tensor_copy(mh[:, 4, 0, 1:8], mlo[:, 4, 0:7])
            nc.vector.tensor_copy(mh[:, 4, 1:64, :], mlo[:, 4, 7:511].rearrange("k (g c) -> k g c", c=8))
            nc.vector.memset(mhis[i][:, 0, 8:40:8], 0.0)    # cl=0   singles
            nc.vector.memset(mhis[i][:, 63, 15:47:8], 0.0)  # cl=511 singles
            _products(nc, mh, R)

        def warmup_phase(n):
            # zero-weight matmuls into the gram PSUM: keeps the PE HAM busy
            # so the gram bursts run at 2.4 GHz; contributes exactly 0.
            for k in range(n):
                nc.tensor.matmul(ps[:, :], wz[:, :], identt[:, 0:128],
                                 start=(k == 0), stop=False)

        def gram_phase(i, last):
            K = KS[i]
            for g in range(64):
                nc.tensor.matmul(
                    ps[:, :],
                    mhis[i][0:K, g, :],
                    mlos[i][0:K, :, 8 * g:8 * g + 8],
                    start=False,
                    stop=(last and g == 63),
                )

        # tile 1 first: its shift is self-contained, so its gram unblocks
        # early; tile 0's shift needs z-tile-1 row 0 (the selector matmul)
        z_phase(1)
        z_phase(0)
        shift_phase(1)
        warmup_phase(30)
        shift_phase(0)
        mono_phase(1)
        gram_phase(1, False)
        mono_phase(0)
        gram_phase(0, True)

        gout = mpool.tile([128, 128], f32, tag="gout")
        nc.scalar.activation(gout[:, :], ps[:, :], Copy)
        nc.sync.dma_start(gram[:, :], gout[:, :])

    nc.compile()
    return nc


def _walsh16():
    sgn = 2.0 * ((np.arange(16)[:, None] >> np.arange(4)[None, :]) & 1) - 1.0
    w = np.ones((16, 16))
    for k in range(16):
        for s in range(16):
            v = 1.0
            for p in range(4):
                if s >> p & 1:
                    v *= sgn[k, p]
            w[k, s] = v
    return w


def _postprocess(grams):
    """grams: 8x [128,128] f32 -> [4,256,1,1]."""
    perm = np.argsort(PLANE_SUBSET)  # subset-index -> plane-index
    w16 = _walsh16()
    hi_sign = np.array([(-1.0) ** bin(s).count("1") for s in range(16)])
    out = np.zeros((4, 256), np.float64)
    for b in range(4):
        g16 = np.zeros((16, 16))
        for half in range(2):
            gr = grams[2 * b + half].astype(np.float64).reshape(16, 8, 16, 8)
            g = np.einsum("tgsg->st", gr)          # sum the 8 diagonal blocks
            g16 += g[np.ix_(perm, perm)]           # plane order -> subset order
        g16 *= hi_sign[None, :]                    # SA planes hold -z_{p+4}
        g16[0, 0] = float(HP * WP)                 # ones*ones: exact pixel count
        hmat = 2.0 ** -8 * (w16 @ g16 @ w16.T)     # [klo, khi]
        out[b] = hmat.T.reshape(256)               # k = klo + 16*khi
    return out.astype(np.float32).reshape(4, 256, 1, 1)


def _ident_np():
    import ml_dtypes
    a = np.zeros((128, 256), dtype=np.float32)
    for m in range(127):
        a[m + 1, m] = 1.0          # subdiagonal: out[m] = z[m+1]
    a[0, 128 + 127] = 1.0          # selector: out[127] = other-tile z[0]
    return a.astype(ml_dtypes.bfloat16)


def kernel(x, diff_hardness, output_hardness):
    global last_results
    from concourse.bass_utils import run_bass_kernel_spmd

    x = np.asarray(x, np.float32)
    dh = float(np.asarray(diff_hardness))
    oh = float(np.asarray(output_hardness))

    key = (dh, oh)
    if key not in _PROGRAM_CACHE:
        _PROGRAM_CACHE[key] = _build_program(dh, oh)
    nc = _PROGRAM_CACHE[key]

    ident = _ident_np()
    in_maps = []
    for core in range(8):
        b, half = divmod(core, 2)
        r0 = 0 if half == 0 else 255
        in_maps.append({
            "xs": np.ascontiguousarray(x[b, 0, r0:r0 + NROWS_SLICE, :]),
            "ident": ident,
        })

    trace = bool(int(os.environ.get("KERNEL_TRACE", "0")))
    res = run_bass_kernel_spmd(nc, in_maps, core_ids=list(range(8)), trace=trace)
    last_results = res
    grams = [res.results[c]["gram"] for c in range(8)]
    return _postprocess(grams)


# revision 11
# speedup vs baseline: 1.8826x; 1.0171x over previous
"""DiffLBP soft-histogram kernel for Trainium2 (8 NeuronCores).

Math: the per-pixel softmax over 256 LBP patterns factorizes exactly into a
product of 8 independent Bernoullis with q_p = 1/2 (1 + z_p),
z_p = tanh((oh/2)*tanh(dh*d_p)).  The histogram is a 16x16 Gram matrix of
z-monomials (4 low bits x 4 high bits) pushed through a constant Walsh +-1
transform (host).  Antipodal offsets give z_{p+4}(r,c) = -z_p((r,c)+off), so
only 4 z planes are computed; the "hi" side needs (row+1, col+dx) shifted
copies: both shifts are done by TensorE (subdiagonal-identity matmul with
col-offset APs into PSUM), drained to SBUF by one ScalarE copy per tile.

Device program per core (one batch b, one 255-row half; tile 1 first):
  z phase (x2 tiles of 128 rows): SWDGE DMA loads XA/XB row-windows cast to
    bf16; DVE computes the 4 diffs; ScalarE does two tanh passes, writing
    the z singles straight into the plane-major mlo tile (planes 1..4).
  shift (x2): TensorE multiplies z planes 0..2 by a subdiagonal identity
    with per-plane column offsets -> PSUM holds the row+col shifted hi
    singles (tile 0 also accumulates a selector matmul that injects
    z-tile-1 row 0 into row 127); ScalarE drains PSUM -> mhi planes 1..3.
  mono phase (x2): both mlo and mhi are plane-major [128, 16, 512]; the 11
    composite monomial planes are built by 4 batched DVE multiplies + 1
    GPSIMD multiply ({03}, which needs only singles so it runs early).
  gram phase (x2): 64 matmuls accumulate into PSUM (strided lhsT selects
    16 planes x 8 cols).  A zero-weight matmul warmup stream keeps the PE
    HAM un-throttled before the gram bursts.  Tile 0 runs K=128 (the
    straddle row center 128 is included on-device), tile 1 K=127 (its row
    127 is the neighbouring core's center).  No host boundary fix.
"""

import os
import numpy as np
from contextlib import ExitStack

H = W = 512
HP = WP = 510          # valid center rows/cols
NROWS_SLICE = 257      # input rows per core slice

# plane slot -> subset bitmask of {z0,z1,z2,z3}; chosen so the 11 composite
# planes are produced by 5 batched multiplies (see _products)
PLANE_SUBSET = [0b0000,
                0b0001, 0b0010, 0b0100, 0b1000,   # 1..4:   z0 z1 z2 z3
                0b0011, 0b0110, 0b1100,           # 5..7:   {01} {12} {23}
                0b0101, 0b1010,                   # 8..9:   {02} {13}
                0b0111, 0b1110, 0b1111,           # 10..12: {012} {123} {0123}
                0b1011, 0b1101,                   # 13..14: {013} {023}
                0b1001]                           # 15:     {03}

_PROGRAM_CACHE = {}
last_results = None  # BassKernelResults of the most recent run (for test harness)


def _products(nc, m, R):
    """Emit the 11 composite monomial planes from singles (planes 1..4) of a
    plane-major view m[[part], 16, W]; writes planes 5..15 on R partitions.
    The {03} plane needs only singles, so it goes to GPSIMD early."""
    nc.gpsimd.tensor_mul(m[:R, 15:16], m[:R, 1:2], m[:R, 4:5])      # 03
    nc.vector.tensor_mul(m[:R, 5:8], m[:R, 1:4], m[:R, 2:5])        # 01 12 23
    nc.vector.tensor_mul(m[:R, 8:10], m[:R, 1:3], m[:R, 3:5])       # 02 13
    nc.vector.tensor_mul(m[:R, 10:13], m[:R, 5:8], m[:R, 3:6])      # 012 123 0123
    nc.vector.tensor_mul(m[:R, 13:15], m[:R, 9:7:-1], m[:R, 1:5:3]) # 013 023


def _build_program(dh: float, oh: float):
    import concourse.bacc as bacc
    import concourse.tile as tile
    from concourse import mybir
    import concourse.bass as bass

    f32 = mybir.dt.float32
    bf16 = mybir.dt.bfloat16
    Tanh = mybir.ActivationFunctionType.Tanh
    Copy = mybir.ActivationFunctionType.Copy

    nc = bacc.Bacc("TRN2", target_bir_lowering=False, debug=False)
    xs_t = nc.dram_tensor("xs", [NROWS_SLICE, W], f32, kind="ExternalInput")
    id_t = nc.dram_tensor("ident", [128, 256], bf16, kind="ExternalInput")
    gram = nc.dram_tensor("gram", [128, 128], f32, kind="ExternalOutput").ap()

    with tile.TileContext(nc) as tc, ExitStack() as ctx:
        xpool = ctx.enter_context(tc.tile_pool(name="x", bufs=2))
        dpool = ctx.enter_context(tc.tile_pool(name="d", bufs=2))
        tpool = ctx.enter_context(tc.tile_pool(name="t", bufs=1))
        mpool = ctx.enter_context(tc.tile_pool(name="m", bufs=1))
        ppool = ctx.enter_context(
            tc.tile_pool(name="ps", bufs=1, space=bass.MemorySpace.PSUM))

        # x loads first (they gate everything); SWDGE casts f32 -> bf16
        xts = {}
        for i in (1, 0):
            xt = xpool.tile([128, 2, W], bf16, name=f"xt{i}", tag=f"xt{i}")
            src = bass.AP(xs_t, 128 * i * W, [[W, 128], [W, 2], [1, W]])
            nc.gpsimd.dma_start(xt[:], src)
            xts[i] = xt

        # shifted-identity weights for the TensorE partition shift
        identt = mpool.tile([128, 256], bf16, tag="identt")
        nc.sync.dma_start(identt[:, :], id_t.ap())

        # trigger the tanh ACT table load immediately (overlaps the X DMAs)
        warm = mpool.tile([1, 8], f32, tag="warm")
        nc.vector.memset(warm[:, :], 0.0)
        nc.scalar.activation(warm[:, :], warm[:, :], Tanh)

        # zero stationary for the PE HAM warmup (contributes 0 to the gram)
        wz = mpool.tile([128, 128], bf16, tag="wz")
        nc.vector.memset(wz[:, :], 0.0)

        ps = ppool.tile([128, 128], f32, tag="ps")
        pshift = {i: ppool.tile([128, 3, W], f32, name=f"pshift{i}",
                                tag=f"pshift{i}") for i in (0, 1)}
        mlos, mhis = {}, {}
        KS = {0: 128, 1: 127}   # gram contraction depth per tile

        # mlo: plane-major (the gram's MOVING operand tolerates 2 free
        # dims); z singles live in planes 1..4.  mhi: chunk-major [128, 64
        # chunks, 16 planes x 8 cols] (the STATIONARY needs 1 free dim).
        mhv = {}
        for i in (1, 0):
            mlos[i] = mpool.tile([128, 16, W], bf16, name=f"mlo{i}", tag=f"mlo{i}")
            mhis[i] = mpool.tile([128, 64, 128], bf16, name=f"mhi{i}", tag=f"mhi{i}")
            mhv[i] = mhis[i][:].rearrange("k g (s c) -> k s g c", c=8)
            nc.gpsimd.memset(mlos[i][:, 0, :], 1.0)
            nc.gpsimd.memset(mhis[i][:, :, 0:8], 1.0)

        def z_phase(i):
            xt = xts[i]
            xa, xb = xt[:, 0, :], xt[:, 1, :]
            d = dpool.tile([128, 4, W], bf16, name=f"d{i}", tag=f"d{i}")
            # d_p[cl] = X_{dy}[cl+dx] - XB[cl]   (cl = x-col = center_col + 1)
            nc.vector.tensor_sub(d[:, 0, 1:512], xa[:, 0:511], xb[:, 1:512])   # (-1,-1)
            nc.vector.tensor_sub(d[:, 1, 0:512], xa[:, 0:512], xb[:, 0:512])   # (-1, 0)
            nc.vector.tensor_sub(d[:, 2, 0:511], xa[:, 1:512], xb[:, 0:511])   # (-1,+1)
            nc.vector.tensor_sub(d[:, 3, 0:511], xb[:, 1:512], xb[:, 0:511])   # ( 0,+1)

            t = tpool.tile([128, 4, W], f32, name=f"t{i}", tag="t")
            nc.scalar.activation(t[:, :, :], d[:, :, :], Tanh, scale=float(dh))
            # z singles straight into the plane-major mlo tile
            nc.scalar.activation(mlos[i][:, 1:5, :], t[:, :, :], Tanh,
                                 scale=float(oh) / 2.0)

        # col windows for the shift matmuls: plane p shifts cols by
        # dx'_p = (+1, 0, -1); out col c reads z col c+dx'
        SHIFT_WIN = [  # (out_lo, out_hi, in_lo, in_hi)
            (0, 511, 1, 512),
            (0, 512, 0, 512),
            (1, 512, 0, 511),
        ]

        def shift_phase(i):
            # pshift[i][rl, p, c] = z_p(row rl+1, col c+dx') via TensorE;
            # tile 0 row 127 = z-tile-1 row 0 via the selector weights.
            for p, (ol, ohi, il, ihi) in enumerate(SHIFT_WIN):
                nc.tensor.matmul(pshift[i][:, p, ol:ohi], identt[:, 0:128],
                                 mlos[i][:, 1 + p, il:ihi],
                                 start=True, stop=(i == 1))
            if i == 0:
                for p, (ol, ohi, il, ihi) in enumerate(SHIFT_WIN):
                    nc.tensor.matmul(pshift[0][:, p, ol:ohi], identt[:, 128:256],
                                     mlos[1][:, 1 + p, il:ihi],
                                     start=False, stop=True)

        def mono_phase(i):
            mlo, mh = mlos[i], mhv[i]
            R = KS[i]
            _products(nc, mlo[:], 128)
            # invalid edge columns: zero ALL lo planes at cl=0 and cl=511
            # (after products; kills the unwritten-d edge garbage too)
            nc.vector.memset(mlo[:, :, 0:512:511], 0.0)
            # hi singles: drain the shift PSUM (planes 1..3, already row+col
            # shifted) into the chunk-major slots; plane 4 = z_3 col-shifted
            # by -1 from mlo plane 4 (one copy; src col -1 lands on the cl=0
            # slot, which the edge memset below overwrites)
            psr = pshift[i][0:R, :, :].rearrange("k p (g c) -> k p g c", c=8)
            p4src = (mlo[:, 3:5, :].rearrange("k p c -> k (p c)")[:, 511:1023]
                     .rearrange("k (g c) -> k g c", c=8))
            if i == 1:
                nc.vector.tensor_copy(mh[0:R, 1:4, :, :], psr)
            else:
                nc.scalar.activation(mh[0:R, 1:4, :, :], psr, Copy)
            nc.vector.tensor_copy(mh[:, 4, :, :], p4src)
            nc.vector.memset(mhis[i][:, 0, 8:40:8], 0.0)    # cl=0   singles
            nc.vector.memset(mhis[i][:, 63, 15:47:8], 0.0)  # cl=511 singles
            _products(nc, mh, R)

        def warmup_phase(n):
            # zero-weight matmuls into the gram PSUM: keeps the PE HAM busy
            # so the gram bursts run at 2.4 GHz; contributes exactly 0.
            for k in range(n):
                nc.tensor.matmul(ps[:, :], wz[:, :], mlos[1][:, 1, 0:128],
                                 start=(k == 0), stop=False)

        def gram_phase(i, last):
            K = KS[i]
            for g in range(64):
                nc.tensor.matmul(
                    ps[:, :],
                    mhis[i][0:K, g, :],
                    mlos[i][0:K, :, 8 * g:8 * g + 8],
                    start=False,
                    stop=(last and g == 63),
                )

        # tile 1 first: its shift is self-contained, so its gram unblocks
        # early; tile 0's shift needs z-tile-1 row 0 (the selector matmul)
        z_phase(1)
        z_phase(0)
        shift_phase(1)
        warmup_phase(30)
        shift_phase(0)
        mono_phase(1)
        gram_phase(1, False)
        mono_phase(0)
        gram_phase(0, True)

        gout = mpool.tile([128, 128], f32, tag="gout")
        nc.scalar.activation(gout[:, :], ps[:, :], Copy)
        nc.sync.dma_start(gram[:, :], gout[:, :])

    nc.compile()
    return nc


def _walsh16():
    sgn = 2.0 * ((np.arange(16)[:, None] >> np.arange(4)[None, :]) & 1) - 1.0
    w = np.ones((16, 16))
    for k in range(16):
        for s in range(16):
            v = 1.0
            for p in range(4):
                if s >> p & 1:
                    v *= sgn[k, p]
            w[k, s] = v
    return w


def _postprocess(grams):
    """grams: 8x [128,128] f32 -> [4,256,1,1]."""
    perm = np.argsort(PLANE_SUBSET)  # subset-index -> plane-index
    w16 = _walsh16()
    hi_sign = np.array([(-1.0) ** bin(s).count("1") for s in range(16)])
    out = np.zeros((4, 256), np.float64)
    for b in range(4):
        g16 = np.zeros((16, 16))
        for half in range(2):
            gr = grams[2 * b + half].astype(np.float64).reshape(16, 8, 16, 8)
            g = np.einsum("tgsg->st", gr)          # sum the 8 diagonal blocks
            g16 += g[np.ix_(perm, perm)]           # plane order -> subset order
        g16 *= hi_sign[None, :]                    # SA planes hold -z_{p+4}
        g16[0, 0] = float(HP * WP)                 # ones*ones: exact pixel count
        hmat = 2.0 ** -8 * (w16 @ g16 @ w16.T)     # [klo, khi]
        out[b] = hmat.T.reshape(256)               # k = klo + 16*khi
    return out.astype(np.float32).reshape(4, 256, 1, 1)


def _ident_np():
    import ml_dtypes
    a = np.zeros((128, 256), dtype=np.float32)
    for m in range(127):
        a[m + 1, m] = 1.0          # subdiagonal: out[m] = z[m+1]
    a[0, 128 + 127] = 1.0          # selector: out[127] = other-tile z[0]
    return a.astype(ml_dtypes.bfloat16)


def kernel(x, diff_hardness, output_hardness):
    global last_results
    from concourse.bass_utils import run_bass_kernel_spmd

    x = np.asarray(x, np.float32)
    dh = float(np.asarray(diff_hardness))
    oh = float(np.asarray(output_hardness))

    key = (dh, oh)
    if key not in _PROGRAM_CACHE:
        _PROGRAM_CACHE[key] = _build_program(dh, oh)
    nc = _PROGRAM_CACHE[key]

    ident = _ident_np()
    in_maps = []
    for core in range(8):
        b, half = divmod(core, 2)
        r0 = 0 if half == 0 else 255
        in_maps.append({
            "xs": np.ascontiguousarray(x[b, 0, r0:r0 + NROWS_SLICE, :]),
            "ident": ident,
        })

    trace = bool(int(os.environ.get("KERNEL_TRACE", "0")))
    res = run_bass_kernel_spmd(nc, in_maps, core_ids=list(range(8)), trace=trace)
    last_results = res
    grams = [res.results[c]["gram"] for c in range(8)]
    return _postprocess(grams)
